# revision 55
# baseline (speedup 1.0000x reference)
# kernel.py -- Trainium2 Bass kernel for nn_BasicTransformerBlock (sparse_attention)
# Self-contained: accepts FULL inputs, shards over 8 NeuronCores internally.
#
# Sharding: core = b*4 + qi handles tokens [b, qi*512:(qi+1)*512] (b in {0,1}).
# Each core redundantly computes its batch's full K/V (no collectives).
#
# v4 (all-bf16 numerics):
#  - u-trick: task-attn scores via u_h = tq_h @ Wtk[block_h,:] + per-token DVE
#    dots against raw task_feat -- the 5 tk projections never materialize.
#  - the post-attention "front" (Wo -> hs1 -> LN -> tq -> u -> dots -> softmax)
#    is split by q-halves; half 0 runs concurrently with attention pass 1.
#  - tv projections accumulate into tout straight from PSUM (t-outer loop).
#  - bf16 recip broadcast (1 cyc/row), fused multiply-reduce dots, PSUM
#    evacuations split Act/DVE, SBUF-only elementwise on GpSimd.
import math
import numpy as np
import ml_dtypes

import concourse.bass as bass
import concourse.mybir as mybir
import concourse.tile as tile
from concourse import bacc
from concourse.bass_utils import run_bass_kernel_spmd
from concourse.masks import make_identity

BF = ml_dtypes.bfloat16
B, S, C, H, DH, T, NA = 2, 2048, 640, 8, 80, 5, 2
DHT = C // NA            # 320
N_CORES = 8
QS = (B * S) // N_CORES  # 512 query tokens per core
QW = QS // 2             # 256 q per attention pass
KC = S // 128            # 16 k sub-chunks
CI = C // 128            # 5 c chunks
MI = QS // 128           # 4 token tiles
EPS = 1e-5
F32 = mybir.dt.float32
BF16 = mybir.dt.bfloat16

TRACE = False            # test.py flips this for profiling runs
ABLATE = 0               # 2: attention+Wo only; 1: no back phase; 0: full
REPS = 1                 # repeat kernel body inside the NEFF (timing slope)
_CACHE = {}


def _build():
    nc = bacc.Bacc("TRN2", target_bir_lowering=False, debug=False,
                   num_devices=N_CORES)
    d = {}

    def din(name, shape, dt=BF16):
        d[name] = nc.dram_tensor(name, shape, dt, kind="ExternalInput").ap()

    din("hsT", [C, S])                       # core's batch, transposed, rotated
    din("tfT", [T, C, QS])                   # core's task_feat slice, transposed
    din("tfn", [T, QS, C])                   # core's task_feat slice, natural
    din("wqT", [C, C])
    din("wkT", [C, C])
    din("wvT", [C, C])
    din("wo_pad", [DH + 1, H, C])            # row0 zero (denominator row)
    din("wtqT", [C, C])                      # g_q-folded, transposed
    din("wtk_nat", [C, C])                   # g_k-folded per column, natural
    din("wtvT", [C, C])                      # g_v-folded, transposed
    din("wotT", [C, C])
    din("wsums", [2, C])                     # colsums of wtqT / wtvT
    din("bo", [CI, 128], F32)
    din("bot", [CI, 128], F32)
    outT = nc.dram_tensor("outT", [C, QS], F32, kind="ExternalOutput").ap()

    with tile.TileContext(nc) as tc:
        for _ in range(REPS):
            _emit(tc, d, outT)
    nc.compile()
    return nc


def _emit(tc, d, outT):
    nc = tc.nc
    import contextlib
    AF = mybir.ActivationFunctionType
    MUL, ADD = mybir.AluOpType.mult, mybir.AluOpType.add
    ctx = contextlib.ExitStack()
    with ctx:
        consts = ctx.enter_context(tc.tile_pool(name="consts", bufs=1))
        persist = ctx.enter_context(tc.tile_pool(name="persist", bufs=1))
        lanes = ctx.enter_context(tc.tile_pool(name="lanes", bufs=1))
        lrot = ctx.enter_context(tc.tile_pool(name="lrot", bufs=3))
        main = ctx.enter_context(tc.tile_pool(name="main", bufs=1))

        # ---------- constants ----------
        ones_bf = consts.tile([128, 1], BF16, tag="ones_bf")
        nc.vector.memset(ones_bf[:], 1.0)
        ones_row_bf = consts.tile([1, 128], BF16, tag="ones_row_bf")
        nc.vector.memset(ones_row_bf[:], 1.0)
        one1_f32 = consts.tile([1, 1], F32, tag="one1")
        nc.vector.memset(one1_f32[:], 1.0)
        zrow_bf = consts.tile([1, 512], BF16, tag="zrow_bf")
        nc.vector.memset(zrow_bf[:], 0.0)
        id_bf = consts.tile([128, 128], BF16, tag="id_bf")
        make_identity(nc, id_bf[:])
        eps_t = consts.tile([1, 1], F32, tag="eps")
        nc.vector.memset(eps_t[:], EPS)
        eps_f = consts.tile([128, 1], F32, tag="eps_f")
        nc.vector.memset(eps_f[:], EPS)
        bo_sb = consts.tile([128, CI], F32, tag="bo")
        nc.sync.dma_start(bo_sb[:], d["bo"].rearrange("c p -> p c"))
        bot_sb = consts.tile([128, CI], F32, tag="bot")
        nc.sync.dma_start(bot_sb[:], d["bot"].rearrange("c p -> p c"))
        wsums_sb = consts.tile([1, 2, C], BF16, tag="wsums")
        nc.sync.dma_start(wsums_sb[:], d["wsums"][None, :, :])

        # ---------- persistent state ----------
        hs1T = persist.tile([128, CI, QS], F32, tag="hs1T")
        hs1_bf = persist.tile([128, CI, QS], BF16, tag="hs1_bf")
        oT = persist.tile([DH + 1, H, QS], BF16, tag="oT")
        recipP = persist.tile([1, H, QW], BF16, tag="recipP")
        rstd0 = lanes.tile([1, QS], F32, tag="rstd0")
        negm0 = lanes.tile([1, QS], F32, tag="negm0")
        negm_bf0 = lanes.tile([1, QS], BF16, tag="negm_bf0")
        mu_t = lanes.tile([1, QS], F32, tag="mu")
        msq_t = lanes.tile([1, QS], F32, tag="msq")

        # ---------- main tiles ----------
        KT = main.tile([DH, H, S], BF16, tag="KT")
        QT = main.tile([DH, H, QS], BF16, tag="QT")
        Vs = main.tile([128, KC, H, DH + 1], BF16, tag="Vs")
        nc.gpsimd.memset(Vs[:, :, :, 0:1], 1.0)
        mvn = main.tile([128, MI, T, 2], F32, tag="mvn")
        wos = main.tile([DH + 1, H, C], BF16, tag="wos")
        attn = main.tile([128, MI, NA, T], F32, tag="attn")
        cneg = main.tile([128, MI, NA], F32, tag="cneg")
        wtvs = main.tile([128, CI, C], BF16, tag="wtvs")
        wots = main.tile([128, CI, C], BF16, tag="wots")
        tfTr = main.tile([128, 2, CI, QW], BF16, tag="tfTr")

        inv_sqrt_dh = 1.0 / math.sqrt(DH)

        def proj(ps, lhs_t, rhs_t, extra=None):
            for j in range(CI):
                nc.tensor.matmul(ps, lhs_t(j), rhs_t(j),
                                 start=(j == 0),
                                 stop=(j == CI - 1 and extra is None),
                                 skip_group_check=(extra is not None))
            if extra is not None:
                extra()

        # ============ phase A+B: projections + attention ============
        with tc.tile_pool(name="po", bufs=1, space="PSUM") as po, \
             tc.tile_pool(name="ptp", bufs=2) as ptp:
            ob0 = [po.tile([128, 2 * QW], F32, tag=f"ob{j}", name=f"ob{j}")
                   for j in range(4)]

            def seed_obanks(obs):
                # one start=True zero matmul per bank: the PSUM zero-region is
                # coarser than a head's half-bank, so per-head start=True
                # would clobber the sibling head's accumulation.
                for j in range(4):
                    nc.tensor.matmul(obs[j][0:DH + 1, :],
                                     zrow_bf[0:1, 0:DH + 1],
                                     zrow_bf[0:1, 0:2 * QW],
                                     start=True, stop=False,
                                     skip_group_check=True)

            def finish_pass(obs, qsl, pool):
                with nc.allow_low_precision(reason="bf16 recip"):
                    for j in range(4):
                        nc.vector.reciprocal(
                            recipP[0:1, 2 * j:2 * j + 2, :],
                            obs[j][0:1, :].rearrange("p (e q) -> p e q", e=2))
                for h in range(H):
                    j, e = divmod(h, 2)
                    bc = pool.tile([128, QS], F32, tag="pjf", name="pbc")
                    nc.tensor.matmul(bc[:, 0:QW], ones_row_bf[:],
                                     recipP[0:1, h, :],
                                     start=True, stop=True)
                    # DVE reads at most one PSUM operand: evacuate the obank
                    # first, then scale in-place by the PSUM broadcast.
                    nc.vector.tensor_copy(oT[:, h, qsl],
                                          obs[j][0:DH + 1, QW * e:QW * (e + 1)])
                    nc.vector.tensor_mul(oT[:, h, qsl], oT[:, h, qsl],
                                         bc[0:DH + 1, 0:QW])

            def attn_u(u, obs, qsl, pool, ptag):
                pt = ptp.tile([128, 2, H, QW], BF16, tag="pt")
                for h in range(H):
                    scp = pool.tile([128, 2, QW], F32, tag=ptag)
                    for e in range(2):
                        ks = 2 * u + e
                        nc.tensor.matmul(
                            scp[:, e, :],
                            KT[:, h, 128 * ks:128 * (ks + 1)],
                            QT[:, h, qsl],
                            start=True, stop=True, skip_group_check=True)
                    nc.scalar.activation(pt[:, :, h, :], scp[:], AF.Exp,
                                         scale=inv_sqrt_dh)
                for h in range(H):
                    j, e = divmod(h, 2)
                    for ee in range(2):
                        nc.tensor.matmul(
                            obs[j][0:DH + 1, QW * e:QW * (e + 1)],
                            Vs[:, 2 * u + ee, h, :],
                            pt[:, ee, h, :],
                            start=False,
                            stop=(u == KC // 2 - 1 and ee == 1),
                            skip_group_check=True)

            with tc.tile_pool(name="inner", bufs=1) as inner, \
                 tc.tile_pool(name="psc", bufs=2, space="PSUM") as psc, \
                 tc.tile_pool(name="pj", bufs=2, space="PSUM") as pj:
                wqs = inner.tile([128, CI, C], BF16, tag="wqs")
                nc.sync.dma_start(
                    wqs[:], d["wqT"].rearrange("(ci p) i -> p ci i", p=128))
                hsTs = inner.tile([128, CI, S], BF16, tag="hsTs")
                nc.sync.dma_start(
                    hsTs[:, :, 0:QS],
                    d["hsT"].rearrange("(ci p) s -> p ci s", p=128)[:, :, 0:QS])
                wks = inner.tile([128, CI, C], BF16, tag="wks")
                nc.sync.dma_start(
                    wks[:], d["wkT"].rearrange("(ci p) i -> p ci i", p=128))
                nc.sync.dma_start(
                    hsTs[:, :, QS:S],
                    d["hsT"].rearrange("(ci p) s -> p ci s", p=128)[:, :, QS:S])
                wvs = inner.tile([128, CI, C], BF16, tag="wvs")
                nc.sync.dma_start(
                    wvs[:], d["wvT"].rearrange("(ci p) i -> p ci i", p=128))
                nc.sync.dma_start(wos[:], d["wo_pad"])
                tfnr = inner.tile([128, 2, MI, C], BF16, tag="tfnr")
                for t in range(2):  # prefetch task-feat naturals (Pool queue)
                    nc.gpsimd.dma_start(
                        tfnr[:, t, :, :],
                        d["tfn"][t].rearrange("(mi p) c -> p mi c", p=128))

                # ---- Q projection (Act evac; Act idle pre-exp) ----
                for h in range(H):
                    ps = pj.tile([128, QS], F32, tag="pj")
                    proj(ps[0:DH, :],
                         lambda j, h=h: wqs[:, j, DH * h:DH * (h + 1)],
                         lambda j: hsTs[:, j, 0:QS])
                    nc.scalar.copy(QT[:, h, :], ps[0:DH, :])
                seed_obanks(ob0)

                # ---- kc loop: K/V production + pass-0 attention ----
                for kc in range(4):
                    for h in range(H):
                        ps = pj.tile([128, QS], F32, tag="pj", name="psK")
                        proj(ps[0:DH, :],
                             lambda j, h=h: wks[:, j, DH * h:DH * (h + 1)],
                             lambda j, kc=kc: hsTs[:, j, QS * kc:QS * (kc + 1)])
                        dst = KT[:, h, QS * kc:QS * (kc + 1)]
                        if kc == 0:
                            nc.scalar.copy(dst, ps[0:DH, :])
                        else:
                            nc.vector.tensor_copy(dst, ps[0:DH, :])
                    for sc in range(4 * kc, 4 * kc + 4):
                        for nch in range(2):
                            ps = pj.tile([128, QS], F32, tag="pj", name="psV")
                            proj(ps[:, 0:DHT],
                                 lambda j, sc=sc: hsTs[:, j,
                                                       128 * sc:128 * (sc + 1)],
                                 lambda j, nch=nch: wvs[:, j,
                                                        DHT * nch:DHT * (nch + 1)])
                            dst = Vs[:, sc, 4 * nch:4 * (nch + 1), 1:DH + 1]
                            src = ps[:, 0:DHT].rearrange("p (h dh) -> p h dh",
                                                         h=4)
                            if kc == 0:
                                nc.scalar.copy(dst, src)
                            else:
                                nc.vector.tensor_copy(dst, src)
                    for u in (2 * kc, 2 * kc + 1):
                        attn_u(u, ob0, slice(0, QW), psc, "scp")
                    # task-feat LN stats for task kc (and 4 on last block)
                    for t in ([kc] if kc < 3 else [3, 4]):
                        for mi in range(MI):
                            stb = lrot.tile([128, 2, 6], F32, tag="stb")
                            xv = tfnr[:, t % 2, mi, :].rearrange(
                                "p (g c) -> p g c", g=2)
                            for g in range(2):
                                nc.vector.bn_stats(stb[:, g, :], xv[:, g, :])
                            nc.vector.bn_aggr(mvn[:, mi, t, :], stb[:])
                        if t + 2 < T:
                            nc.gpsimd.dma_start(
                                tfnr[:, t % 2, :, :],
                                d["tfn"][t + 2].rearrange(
                                    "(mi p) c -> p mi c", p=128))

            # ---- front pools (reuse inner's space) + front psum ----
            fctx = contextlib.ExitStack()
            pjf = fctx.enter_context(
                tc.tile_pool(name="pjf", bufs=2, space="PSUM"))
            psc1 = fctx.enter_context(
                tc.tile_pool(name="psc1", bufs=2, space="PSUM"))
            frontp = fctx.enter_context(tc.tile_pool(name="frontp", bufs=1))
            wtqs = frontp.tile([128, CI, C], BF16, tag="wtqs")
            nc.sync.dma_start(
                wtqs[:], d["wtqT"].rearrange("(ci p) i -> p ci i", p=128))
            wtks = frontp.tile([128, CI, C], BF16, tag="wtks")
            nc.sync.dma_start(
                wtks[:], d["wtk_nat"].rearrange("(ci p) i -> p ci i", p=128))
            tfnh = frontp.tile([128, 2, 2, C], BF16, tag="tfnh")
            nc.sync.dma_start(
                wtvs[:], d["wtvT"].rearrange("(ci p) i -> p ci i", p=128))
            nc.sync.dma_start(
                wots[:], d["wotT"].rearrange("(ci p) i -> p ci i", p=128))
            tvr = frontp.tile([128, 3, 2, C], BF16, tag="tvr")
            tout = frontp.tile([128, MI, C], BF16, tag="tout")
            toutT = frontp.tile([128, CI, QS], BF16, tag="toutT")
            outT_sb = frontp.tile([128, CI, QW], F32, tag="outT_sb")
            wsv_b = frontp.tile([128, C], BF16, tag="wsv_b")
            tqT = frontp.tile([128, CI, QS], BF16, tag="tqT")
            u_sb = frontp.tile([128, NA, MI, C], BF16, tag="u_sb")
            su = frontp.tile([128, MI, NA], F32, tag="su")
            r1n = frontp.tile([128, MI, 1], F32, tag="r1n")
            rstdn = frontp.tile([128, MI, T], F32, tag="rstdn")
            mnegn = frontp.tile([128, MI, T], F32, tag="mnegn")
            scores = frontp.tile([128, MI, NA, T], F32, tag="scores")
            esc = frontp.tile([128, MI, NA, T], F32, tag="esc")
            den = frontp.tile([128, MI, NA], F32, tag="den")

            def front(hf):
                # post-attention chain for q-half hf (tokens qsl, mi in mis)
                qsl = slice(QW * hf, QW * (hf + 1))
                mis = (2 * hf, 2 * hf + 1)
                # Wo -> hs1 (this half)
                for ci in range(CI):
                    ps = pjf.tile([128, QS], F32, tag="pjf", name="pwo")
                    for h in range(H):
                        nc.tensor.matmul(ps[:, 0:QW],
                                         wos[:, h, 128 * ci:128 * (ci + 1)],
                                         oT[:, h, qsl],
                                         start=(h == 0), stop=(h == H - 1))
                    nc.vector.tensor_scalar(out=hs1T[:, ci, qsl],
                                            in0=ps[:, 0:QW],
                                            scalar1=bo_sb[:, ci:ci + 1],
                                            scalar2=None, op0=ADD)
                    nc.vector.tensor_copy(hs1_bf[:, ci, qsl], hs1T[:, ci, qsl])
                if ABLATE >= 2:
                    return
                # LN stats for this half (xsq per-ci in a small tile)
                pstat = pjf.tile([128, QS], F32, tag="pjf", name="pstat")
                stm, sts = pstat[0:1, 0:QW], pstat[64:65, 0:QW]
                for ci in range(CI):
                    nc.tensor.matmul(stm, ones_bf[:], hs1_bf[:, ci, qsl],
                                     start=(ci == 0), stop=(ci == CI - 1),
                                     skip_group_check=True)
                for ci in range(CI):
                    xsq1 = lrot.tile([128, QW], BF16, tag="xsq1", bufs=2)
                    nc.vector.tensor_mul(xsq1[:], hs1_bf[:, ci, qsl],
                                         hs1_bf[:, ci, qsl])
                    nc.tensor.matmul(sts, ones_bf[:], xsq1[:],
                                     start=(ci == 0), stop=(ci == CI - 1),
                                     skip_group_check=True)
                nc.vector.tensor_scalar(out=mu_t[0:1, qsl], in0=stm,
                                        scalar1=1.0 / C, scalar2=None, op0=MUL)
                nc.vector.tensor_scalar(out=msq_t[0:1, qsl], in0=sts,
                                        scalar1=1.0 / C, scalar2=None, op0=MUL)
                nc.vector.tensor_mul(negm0[0:1, qsl], mu_t[0:1, qsl],
                                     mu_t[0:1, qsl])
                nc.vector.tensor_sub(msq_t[0:1, qsl], msq_t[0:1, qsl],
                                     negm0[0:1, qsl])
                nc.vector.tensor_scalar(out=negm0[0:1, qsl],
                                        in0=mu_t[0:1, qsl],
                                        scalar1=-1.0, scalar2=None, op0=MUL)
                nc.scalar.copy(negm_bf0[0:1, qsl], negm0[0:1, qsl])
                # tqT projection for this half (rank-1 -mean fold on PE)
                for jc in range(CI):
                    ps = pjf.tile([128, QS], F32, tag="pjf", name="pjq")

                    def fold(ps=ps, jc=jc):
                        nc.tensor.matmul(
                            ps[:, 0:QW],
                            wsums_sb[0:1, 0, 128 * jc:128 * (jc + 1)],
                            negm_bf0[0:1, qsl], start=False, stop=True,
                            skip_group_check=True)
                    proj(ps[:, 0:QW],
                         lambda ci, jc=jc: wtqs[:, ci, 128 * jc:128 * (jc + 1)],
                         lambda ci: hs1_bf[:, ci, qsl], extra=fold)
                    nc.vector.tensor_copy(tqT[:, jc, qsl], ps[:, 0:QW])
                # u projections (head h = tqT partition rows [320h, 320h+320))
                DCH = {0: [(0, 0, 128), (1, 0, 128), (2, 0, 64)],
                       1: [(2, 64, 128), (3, 0, 128), (4, 0, 128)]}
                for h in range(NA):
                    for mi in mis:
                        for nch in range(2):
                            nsl = slice(DHT * nch, DHT * (nch + 1))
                            ps = pjf.tile([128, QS], F32, tag="pjf",
                                          name="pju")
                            for i, (jc, p0, p1) in enumerate(DCH[h]):
                                nc.tensor.matmul(
                                    ps[:, 0:DHT],
                                    tqT[p0:p1, jc, 128 * mi:128 * (mi + 1)],
                                    wtks[p0:p1, jc, nsl],
                                    start=(i == 0), stop=(i == 2))
                            if hf == 0:
                                nc.vector.tensor_copy(u_sb[:, h, mi, nsl],
                                                      ps[:, 0:DHT])
                            else:
                                nc.scalar.copy(u_sb[:, h, mi, nsl],
                                               ps[:, 0:DHT])
                        nc.vector.reduce_sum(su[:, mi, h:h + 1],
                                             u_sb[:, h, mi, :][:, None, :],
                                             axis=mybir.AxisListType.X)
                # rstd for this half (Sqrt table shared with rstdn)
                nc.scalar.activation(msq_t[0:1, qsl], msq_t[0:1, qsl],
                                     AF.Sqrt, bias=eps_t[:])
                nc.vector.reciprocal(rstd0[0:1, qsl], msq_t[0:1, qsl])
                nc.scalar.activation(rstdn[:, 2 * hf:2 * hf + 2, :],
                                     mvn[:, 2 * hf:2 * hf + 2, :, 1],
                                     AF.Sqrt, bias=eps_f)
                nc.vector.reciprocal(rstdn[:, 2 * hf:2 * hf + 2, :],
                                     rstdn[:, 2 * hf:2 * hf + 2, :])
                nc.vector.tensor_scalar(out=mnegn[:, 2 * hf:2 * hf + 2, :],
                                        in0=mvn[:, 2 * hf:2 * hf + 2, :, 0],
                                        scalar1=-1.0, scalar2=None, op0=MUL)
                for mi in mis:
                    tp = pjf.tile([128, QS], F32, tag="pjf", name="ptr")
                    nc.tensor.transpose(tp[:, 0:1],
                                        rstd0[0:1, 128 * mi:128 * (mi + 1)],
                                        one1_f32[:])
                    nc.vector.tensor_copy(r1n[:, mi, :], tp[:, 0:1])
                # scores: dots of raw task_feat against u (stream tfn halves)
                scr = lrot.tile([128, NA, 2, C], BF16, tag="ttr_scr", bufs=1)
                for t in range(2):
                    nc.gpsimd.dma_start(
                        tfnh[:, t, :, :],
                        d["tfn"][t].rearrange(
                            "(mi p) c -> p mi c", p=128)[:, 2 * hf:2 * hf + 2, :])
                for t in range(T):
                    # one wide mul + reduce over (h, mi) at once
                    nc.vector.tensor_mul(
                        scr[:],
                        tfnh[:, t % 2, :, :][:, None, :, :].broadcast_to(
                            [128, NA, 2, C]),
                        u_sb[:, :, 2 * hf:2 * hf + 2, :])
                    nc.vector.reduce_sum(
                        scores[:, 2 * hf:2 * hf + 2, :, t].rearrange(
                            "p mi h -> p h mi"),
                        scr[:], axis=mybir.AxisListType.X)
                    if t + 2 < T:
                        nc.gpsimd.dma_start(
                            tfnh[:, t % 2, :, :],
                            d["tfn"][t + 2].rearrange(
                                "(mi p) c -> p mi c",
                                p=128)[:, 2 * hf:2 * hf + 2, :])
                # corrections + softmax over t (batched broadcast ops)
                msl = slice(2 * hf, 2 * hf + 2)
                smn = lrot.tile([128, 2, NA, T], F32, tag="smn", bufs=1)
                nc.vector.tensor_mul(
                    smn[:],
                    su[:, msl, :, None].broadcast_to([128, 2, NA, T]),
                    mnegn[:, msl, None, :].broadcast_to([128, 2, NA, T]))
                nc.vector.tensor_add(scores[:, msl, :, :],
                                     scores[:, msl, :, :], smn[:])
                r1rk = lrot.tile([128, 2, T], F32, tag="r1rk")
                nc.vector.tensor_mul(
                    r1rk[:], rstdn[:, msl, :],
                    r1n[:, msl, :].broadcast_to([128, 2, T]))
                nc.vector.tensor_mul(
                    scores[:, msl, :, :], scores[:, msl, :, :],
                    r1rk[:, :, None, :].broadcast_to([128, 2, NA, T]))
                nc.scalar.activation(esc[:, msl, :, :], scores[:, msl, :, :],
                                     AF.Exp, scale=1.0 / math.sqrt(DHT))
                nc.vector.reduce_sum(den[:, msl, :], esc[:, msl, :, :],
                                     axis=mybir.AxisListType.X)
                nc.vector.reciprocal(den[:, msl, :], den[:, msl, :])
                nc.vector.tensor_mul(
                    attn[:, msl, :, :], esc[:, msl, :, :],
                    den[:, msl, :, None].broadcast_to([128, 2, NA, T]))
                nc.vector.tensor_mul(
                    attn[:, msl, :, :], attn[:, msl, :, :],
                    rstdn[:, msl, None, :].broadcast_to([128, 2, NA, T]))
                nc.vector.tensor_mul(
                    smn[:], attn[:, msl, :, :],
                    mnegn[:, msl, None, :].broadcast_to([128, 2, NA, T]))
                nc.vector.reduce_sum(cneg[:, msl, :], smn[:],
                                     axis=mybir.AxisListType.X)

            def back(hf):
                # tv projections + tout accumulation + Wot for q-half hf
                qsl = slice(QW * hf, QW * (hf + 1))
                mis = (2 * hf, 2 * hf + 1)
                if hf == 0:
                    for nch in range(2):
                        nsl = slice(DHT * nch, DHT * (nch + 1))
                        bp = pjf.tile([128, QS], F32, tag="pjf", name="pwb")
                        nc.tensor.matmul(bp[:, 0:DHT], ones_row_bf[:],
                                         wsums_sb[0:1, 1, nsl],
                                         start=True, stop=True)
                        nc.vector.tensor_copy(wsv_b[:, nsl], bp[:, 0:DHT])
                for t in range(2):
                    nc.gpsimd.dma_start(
                        tfTr[:, t, :, :],
                        d["tfT"][t].rearrange(
                            "(ci p) n -> p ci n", p=128)[:, :, qsl])
                for t in range(T):
                    for i, mi in enumerate(mis):
                        for h in range(NA):
                            nsl = slice(DHT * h, DHT * (h + 1))
                            ps = pjf.tile([128, QS], F32, tag="pjf",
                                          name="pjv")
                            proj(ps[:, 0:DHT],
                                 lambda j, t=t, i=i:
                                     tfTr[:, t % 2, j,
                                          128 * i:128 * (i + 1)],
                                 lambda j, nsl=nsl: wtvs[:, j, nsl])
                            nc.scalar.copy(tvr[:, t % 3, i, nsl],
                                           ps[:, 0:DHT])
                            a_sc = attn[:, mi, h, t][:, None]
                            if t == 0:
                                nc.vector.tensor_scalar(
                                    out=tout[:, mi, nsl],
                                    in0=tvr[:, t % 3, i, nsl],
                                    scalar1=a_sc, scalar2=None, op0=MUL)
                            else:
                                nc.vector.scalar_tensor_tensor(
                                    out=tout[:, mi, nsl],
                                    in0=tvr[:, t % 3, i, nsl],
                                    scalar=a_sc, in1=tout[:, mi, nsl],
                                    op0=MUL, op1=ADD)
                    if t + 2 < T:
                        nc.gpsimd.dma_start(
                            tfTr[:, t % 2, :, :],
                            d["tfT"][t + 2].rearrange(
                                "(ci p) n -> p ci n", p=128)[:, :, qsl])
                # tv-side LN mean correction (in-place on tout)
                for mi in mis:
                    for h in range(NA):
                        hsl = slice(DHT * h, DHT * (h + 1))
                        nc.vector.scalar_tensor_tensor(
                            out=tout[:, mi, hsl], in0=wsv_b[:, hsl],
                            scalar=cneg[:, mi, h][:, None],
                            in1=tout[:, mi, hsl], op0=MUL, op1=ADD)
                # transpose this half's tout -> toutT columns
                for ci in range(CI):
                    tpb = pjf.tile([128, QS], BF16, tag="pjf", name="trpb")
                    for i, mi in enumerate(mis):
                        nc.tensor.transpose(
                            tpb[:, 128 * i:128 * (i + 1)],
                            tout[:, mi, 128 * ci:128 * (ci + 1)], id_bf[:])
                    nc.scalar.copy(toutT[:, ci, qsl], tpb[:, 0:QW])
                # Wot + residual + bias for this half
                for ci in range(CI):
                    ps = pjf.tile([128, QS], F32, tag="pjf", name="pjo")
                    for ki in range(CI):
                        nc.tensor.matmul(ps[:, 0:QW],
                                         wots[:, ki, 128 * ci:128 * (ci + 1)],
                                         toutT[:, ki, qsl],
                                         start=(ki == 0), stop=(ki == CI - 1))
                    nc.vector.tensor_add(outT_sb[:, ci, :], ps[:, 0:QW],
                                         hs1T[:, ci, qsl])
                    nc.scalar.activation(outT_sb[:, ci, :],
                                         outT_sb[:, ci, :],
                                         AF.Identity,
                                         bias=bot_sb[:, ci:ci + 1])
                    nc.sync.dma_start(
                        outT.rearrange("(ci p) n -> p ci n",
                                       p=128)[:, ci, qsl],
                        outT_sb[:, ci, :])

            # ---- pre-emit pass-1 scores/exp for u=0,1: fills the PE gap
            # while the DVE finish-pass chain drains (AV deferred until the
            # obanks are recycled) ----
            pts01 = []
            for u in (0, 1):
                pt = ptp.tile([128, 2, H, QW], BF16, tag="pt")
                for h in range(H):
                    scp = psc1.tile([128, 2, QW], F32, tag="scp1")
                    for e in range(2):
                        ks = 2 * u + e
                        nc.tensor.matmul(
                            scp[:, e, :],
                            KT[:, h, 128 * ks:128 * (ks + 1)],
                            QT[:, h, QW:QS],
                            start=True, stop=True, skip_group_check=True)
                    nc.scalar.activation(pt[:, :, h, :], scp[:], AF.Exp,
                                         scale=inv_sqrt_dh)
                pts01.append(pt)

            # ---- finish pass 0 and overlap front(0) with pass 1 ----
            finish_pass(ob0, slice(0, QW), pjf)
            front(0)

            ob1 = [po.tile([128, 2 * QW], F32, tag=f"ob{j}", name=f"ob{j}")
                   for j in range(4)]
            seed_obanks(ob1)
            for u, pt in zip((0, 1), pts01):
                for h in range(H):
                    j, e = divmod(h, 2)
                    for ee in range(2):
                        nc.tensor.matmul(
                            ob1[j][0:DH + 1, QW * e:QW * (e + 1)],
                            Vs[:, 2 * u + ee, h, :],
                            pt[:, ee, h, :],
                            start=False, stop=False, skip_group_check=True)
            for u in range(2, KC // 2):
                attn_u(u, ob1, slice(QW, QS), psc1, "scp1")
            back(0)
            finish_pass(ob1, slice(QW, QS), pjf)
            front(1)
            back(1)
            fctx.close()



def _prep(inputs):
    """Host-side relayout: transposes, casts, folds. No data FLOPs."""
    f32 = np.float32
    hs = np.asarray(inputs["hidden_states"], f32)
    tf = np.asarray(inputs["task_feat"], f32)
    for bn in ("ln_q_b", "ln_k_b", "ln_v_b"):
        if np.abs(np.asarray(inputs[bn], f32)).max() != 0.0:
            raise NotImplementedError("nonzero LayerNorm bias not supported")

    def t_bf(x):
        return np.ascontiguousarray(x.T).astype(BF)

    wqT, wkT, wvT = (t_bf(np.asarray(inputs[k], f32)) for k in ("Wq", "Wk", "Wv"))
    woT = np.ascontiguousarray(np.asarray(inputs["Wo"], f32).T)   # [inner, c]
    wo_pad = np.zeros((DH + 1, H, C), f32)
    for h in range(H):
        wo_pad[1:DH + 1, h, :] = woT[DH * h:DH * (h + 1), :]
    wo_pad = wo_pad.astype(BF)

    g_q = np.asarray(inputs["ln_q_g"], f32)
    g_k = np.asarray(inputs["ln_k_g"], f32)
    g_v = np.asarray(inputs["ln_v_g"], f32)
    wtqT = np.ascontiguousarray(
        np.asarray(inputs["Wtq"], f32).T * g_q[:, None]).astype(BF)
    # u-trick: u_h = tq_h @ (Wtk * g_k[col]) in natural orientation
    wtk_nat = np.ascontiguousarray(
        np.asarray(inputs["Wtk"], f32) * g_k[None, :]).astype(BF)
    wtvT = np.ascontiguousarray(
        np.asarray(inputs["Wtv"], f32).T * g_v[:, None]).astype(BF)
    wotT = t_bf(np.asarray(inputs["Wot"], f32))
    wsums = np.zeros((2, C), f32)
    wsums[0] = wtqT.astype(f32).sum(axis=0)
    wsums[1] = wtvT.astype(f32).sum(axis=0)
    wsums = wsums.astype(BF)
    bo = np.ascontiguousarray(np.asarray(inputs["bo"], f32).reshape(CI, 128))
    bot = np.ascontiguousarray(np.asarray(inputs["bot"], f32).reshape(CI, 128))

    hsT_b = [t_bf(hs[b]) for b in range(B)]        # [C, S] bf16 per batch
    in_maps = []
    for core in range(N_CORES):
        b, qi = divmod(core, 4)
        q0 = QS * qi
        hsT_rot = np.ascontiguousarray(
            np.concatenate([hsT_b[b][:, q0:], hsT_b[b][:, :q0]], axis=1))
        tfT = np.ascontiguousarray(
            tf[:, b, q0:q0 + QS, :].transpose(0, 2, 1)).astype(BF)
        tfn = np.ascontiguousarray(tf[:, b, q0:q0 + QS, :]).astype(BF)
        in_maps.append({"hsT": hsT_rot, "tfT": tfT, "tfn": tfn,
                        "wqT": wqT, "wkT": wkT, "wvT": wvT, "wo_pad": wo_pad,
                        "wtqT": wtqT, "wtk_nat": wtk_nat, "wtvT": wtvT,
                        "wotT": wotT, "wsums": wsums, "bo": bo, "bot": bot})
    return in_maps


def kernel(**inputs):
    in_maps = _prep(inputs)
    if "nc" not in _CACHE:
        _CACHE["nc"] = _build()
    nc = _CACHE["nc"]
    res = run_bass_kernel_spmd(nc, in_maps, core_ids=list(range(N_CORES)),
                               trace=TRACE)
    _CACHE["last_results"] = res
    out = np.empty((B, S, C), np.float32)
    for core in range(N_CORES):
        b, qi = divmod(core, 4)
        q0 = QS * qi
        out[b, q0:q0 + QS, :] = res.results[core]["outT"].T
    return out
